# revision 53
# baseline (speedup 1.0000x reference)
"""Trainium2 Bass kernel for nn_AttentionHead (B=8, S=2048, E=1024, Dh=64).

Sharding: data-parallel over batch B across the 8 NeuronCores (one batch
element per core); W/b replicated; results gathered on host.

Per-core computation, all in "transposed" orientation so no large fp32
matrix ever needs a transpose after the score matmul:
  qkv = x @ W + b; q,k,v = split(qkv); the 1/sqrt(Dh) scale folded into W_q,b_q
  scores^T[k,q] = k^T.T @ q^T          (Dh=64 contraction, PE, fp32r)
  s_m = scores^T * (1-mask^T)          (DVE; bf16 mask, exact 0/1)
  e = exp(s_m)                         (ACT; masked entries -> exp(0)=1,
                                        matching torch masked_fill(mask==1, 0))
  [num^T; Z] = [v | 1]^T @ e           (PE accumulate over k-chunks; the ones
                                        column yields the softmax denominator)
  out = (num * (1/Z))^T                (small PE transpose + DVE reciprocal)

Layout changes: x (8MB) goes through exact fp32 PE transposes; the int32
mask is converted to bf16 t=1-m on GPSIMD and transposed per q-block
either on the PE (first two blocks, whose psum->slab copies land in the
early DVE/ACT slack window) or mostly on the DMA xbar (last two blocks,
keeping copies out of the compute-saturated back half). Matmuls run as
float32r (full PE rate; ~11-bit mantissa) giving ~6e-4 max relative error
end-to-end; set MM_DT = F32 for exact-fp32 matmuls at 4x PE cost.

Cost-model simulated span ~113.5 us/core. Profile shape: DMA-bound to
~60 us (stream gap-free, ends 87.5 us), then DVE/ACT-bound (the 44 us
fp32 mask multiply at DVE's 1x rate plus the 39 us ACT exp pace the
back half), with a ~10 us drain tail.
"""

import os
import sys

sys.path.insert(0, "/opt/trn_rl_repo")

import numpy as np

import concourse.bass as bass
import concourse.tile as tile
from concourse import bacc, mybir
from concourse.masks import make_identity
from concourse.bass_utils import run_bass_kernel_spmd

F32 = mybir.dt.float32
F32R = mybir.dt.float32r
BF16 = mybir.dt.bfloat16
I32 = mybir.dt.int32

B, S, E, DH = 8, 2048, 1024, 64
N_CORES = 8
SCALE = 1.0 / 8.0  # 1/sqrt(DH)

# matmul dtype: F32R = full-rate reduced precision, F32 = exact, 4x slower
MM_DT = F32R

EC = E // 128     # 8   e-chunks
SC = S // 128     # 16  s-chunks (also k-chunks)
QB = S // 512     # 4   q-blocks
KC = SC           # 16  k-chunks per q-block

AF = mybir.ActivationFunctionType
OP = mybir.AluOpType


def build(nc: bass.Bass):
    x_in = nc.dram_tensor("x", [S, E], F32, kind="ExternalInput")
    m_in = nc.dram_tensor("mask", [S, S], I32, kind="ExternalInput")
    w_in = nc.dram_tensor("W", [E, 3 * DH], F32, kind="ExternalInput")
    b_in = nc.dram_tensor("b", [3 * DH], F32, kind="ExternalInput")
    o_out = nc.dram_tensor("out", [S, DH], F32, kind="ExternalOutput")

    trace_sim = bool(os.environ.get("TRN_TRACE_SIM"))
    with tile.TileContext(nc, trace_sim=trace_sim) as tc:
        with (
            tc.tile_pool(name="persist", bufs=1) as persist,
            tc.tile_pool(name="small", bufs=1) as small,
        ):
            # ---- constants / weights -------------------------------------
            ident = persist.tile([128, 128], F32)
            make_identity(nc, ident)
            ident_bf = persist.tile([128, 128], BF16)
            nc.vector.tensor_copy(ident_bf[:], ident[:])

            w_raw = small.tile([128, EC, 3 * DH], F32)
            nc.gpsimd.dma_start(w_raw[:], w_in.rearrange("(o p) d -> p o d", p=128))
            w_sb = persist.tile([128, EC, 3 * DH], MM_DT)
            # fold the 1/sqrt(DH) scale into W_q
            nc.vector.tensor_scalar_mul(w_sb[:, :, 0:DH], w_raw[:, :, 0:DH], SCALE)
            nc.vector.tensor_copy(w_sb[:, :, DH:], w_raw[:, :, DH:])

            b_q_raw = small.tile([64, 1], F32)
            nc.gpsimd.dma_start(b_q_raw[:], b_in[0:64].unsqueeze(-1))
            b_q = persist.tile([64, 1], F32)
            nc.vector.tensor_scalar_mul(b_q[:], b_q_raw[:], SCALE)
            b_k = persist.tile([64, 1], F32)
            nc.gpsimd.dma_start(b_k[:], b_in[64:128].unsqueeze(-1))
            b_v = persist.tile([64, 1], F32)
            nc.gpsimd.dma_start(b_v[:], b_in[128:192].unsqueeze(-1))

            # persistent activations (separate tiles: matmul needs equal base partitions)
            qT = persist.tile([64, S], MM_DT)         # q^T, scale folded
            kT = persist.tile([64, S], MM_DT)         # k^T
            v1 = persist.tile([128, SC, DH + 1], MM_DT)  # v natural + ones col
            ones_col = small.tile([128, 1], F32)
            nc.vector.memset(ones_col[:], 1.0)
            for c in range(SC):
                nc.vector.tensor_copy(v1[:, c, DH : DH + 1], ones_col[:])

            # ---- mask pipeline (spans phase 1 for prefetch) -------------
            from contextlib import ExitStack

            mask_ctx = ExitStack()
            p_m = mask_ctx.enter_context(tc.tile_pool(name="mstage", bufs=4))
            p_tnat = mask_ctx.enter_context(tc.tile_pool(name="tnat", bufs=4))
            p_slab = mask_ctx.enter_context(tc.tile_pool(name="slab", bufs=2))
            ps_tm = mask_ctx.enter_context(
                tc.tile_pool(name="ps_tm", bufs=2, space="PSUM")
            )

            slabs = {}

            def prep(qb, pe_chunks=(3,)):
                """mask rows for q-block -> t^T slab [k_in, k_out, q].

                Chunks in pe_chunks are transposed on the PE instead of the
                DMA xbar (keeps them off the serialized DMA stream)."""
                slab = p_slab.tile([128, SC, 512], BF16)
                m_tiles = []
                for mc in range(4):
                    m_i32 = p_m.tile([128, S], I32)
                    nc.sync.dma_start(
                        m_i32[:],
                        m_in[qb * 512 + mc * 128 : qb * 512 + (mc + 1) * 128, :],
                    )
                    m_tiles.append(m_i32)
                for mc in range(4):
                    t_nat = p_tnat.tile([128, S], BF16)
                    nc.gpsimd.tensor_scalar(
                        t_nat[:], m_tiles[mc][:], -1, 1, OP.mult, OP.add
                    )
                    if mc not in pe_chunks:
                        nc.sync.dma_start_transpose(
                            slab[:, :, mc * 128 : (mc + 1) * 128], t_nat[:]
                        )
                    else:
                        # PE path: offload the 4th transpose from the DMA xbar
                        for g in range(4):
                            pstm = ps_tm.tile([128, 512], BF16, name="pstm")
                            for j4 in range(4):
                                kc_t = g * 4 + j4
                                nc.tensor.transpose(
                                    pstm[:, j4 * 128 : (j4 + 1) * 128],
                                    t_nat[:, kc_t * 128 : (kc_t + 1) * 128],
                                    ident_bf[:],
                                )
                            cp = nc.scalar.copy if (g + mc) % 2 else nc.vector.tensor_copy
                            cp(
                                slab[:, g * 4 : (g + 1) * 4,
                                     mc * 128 : (mc + 1) * 128],
                                pstm[:].rearrange("p (j f) -> p j f", j=4),
                            )
                slabs[qb] = slab

            # ---- phase 1: x^T, qkv (mask prefetch interleaved) ----------
            with (
                tc.tile_pool(name="xnat", bufs=2) as p_xnat,
                tc.tile_pool(name="xT", bufs=2) as p_xT,
                tc.tile_pool(name="vT", bufs=1) as p_vT,
                tc.tile_pool(name="ps_t", bufs=2, space="PSUM") as ps_t,
                tc.tile_pool(name="ps_mm", bufs=2, space="PSUM") as ps_mm,
            ):
                vT = p_vT.tile([64, S], F32)
                for nt in range(QB):
                    sl = slice(nt * 512, (nt + 1) * 512)
                    x_T = p_xT.tile([128, EC, 512], MM_DT)
                    x_nat4 = p_xnat.tile([128, 4, E], F32)
                    for h in range(2):  # two 1MB halves for finer pipelining
                        nc.sync.dma_start(
                            x_nat4[:, h * 2 : (h + 1) * 2, :],
                            x_in[nt * 512 + h * 256 : nt * 512 + (h + 1) * 256, :]
                            .rearrange("(c p) e -> p c e", p=128),
                        )
                    for c4 in range(4):
                        c = nt * 4 + c4
                        for g in range(2):  # 4 transposes -> 1 psum tile
                            pst = ps_t.tile([128, 512], F32, name="pst")
                            for j4 in range(4):
                                j = g * 4 + j4
                                nc.tensor.transpose(
                                    pst[:, j4 * 128 : (j4 + 1) * 128],
                                    x_nat4[:, c4, j * 128 : (j + 1) * 128],
                                    ident[:],
                                )
                            # copy psum -> x_T (rounds to f32r); alternate engines
                            dst = x_T[:, g * 4 : (g + 1) * 4, c4 * 128 : (c4 + 1) * 128]
                            src = pst[:].rearrange("p (j f) -> p j f", j=4)
                            if c % 2 == 0:
                                nc.vector.tensor_copy(dst, src)
                            else:
                                nc.scalar.copy(dst, src)

                    for dst, lo, bias in (
                        (qT, 0, b_q),
                        (kT, DH, b_k),
                        (vT, 2 * DH, b_v),
                    ):
                        ps_p = ps_mm.tile([64, 512], F32, name="ps_p")
                        for j in range(EC):
                            nc.tensor.matmul(
                                ps_p[:],
                                w_sb[:, j, lo : lo + DH],
                                x_T[:, j, :],
                                start=(j == 0),
                                stop=(j == EC - 1),
                            )
                        nc.vector.tensor_scalar_add(dst[:, sl], ps_p[:], bias[:])

                    # v natural for this 512-slice
                    for c4 in range(4):
                        c = nt * 4 + c4
                        psv = ps_t.tile([128, 512], F32, name="pst")
                        nc.tensor.transpose(
                            psv[:, 0:64],
                            vT[:, c * 128 : (c + 1) * 128],
                            ident[0:64, 0:64],
                        )
                        nc.vector.tensor_copy(v1[:, c, 0:DH], psv[:, 0:64])

                # mask prefetch for the first two q-blocks after all x traffic.
                # These two transpose fully on the PE: their psum->slab copies
                # land in the early window where DVE/ACT are idle, and the DMA
                # stream (which gates the last q-block's compute) shortens.
                prep(0, pe_chunks=(0, 1, 2, 3))
                prep(1, pe_chunks=(0, 1, 2, 3))

            # ---- phase 3: attention -------------------------------------
            with (
                tc.tile_pool(name="sm", bufs=4) as p_sm,
                tc.tile_pool(name="e", bufs=4) as p_e,
                tc.tile_pool(name="nz", bufs=2) as p_nz,
                tc.tile_pool(name="osb", bufs=2) as p_o,
                tc.tile_pool(name="ps_s", bufs=3, space="PSUM") as ps_s,
                tc.tile_pool(name="ps_o", bufs=2, space="PSUM") as ps_o,
                tc.tile_pool(name="ps_t2", bufs=1, space="PSUM") as ps_t2,
            ):
                def attend(q0, width, slab):
                    """kc-loop + normalize + store for q columns [q0, q0+width)."""
                    nblk = width // 128
                    po_full = ps_o.tile([DH + 1, 512], F32, name="po")
                    po = po_full[:, :width]
                    for kc in range(KC):
                        pss_full = ps_s.tile([128, 512], F32, name="pss")
                        pss = pss_full[:, :width]
                        nc.tensor.matmul(
                            pss[:],
                            kT[:, kc * 128 : (kc + 1) * 128],
                            qT[:, q0 : q0 + width],
                            start=True,
                            stop=True,
                        )
                        s_m_full = p_sm.tile([128, 512], F32, tag="sm")
                        s_m = s_m_full[:, :width]
                        nc.vector.tensor_tensor(
                            s_m[:], pss[:],
                            slab[:, kc, q0 % 512 : q0 % 512 + width], OP.mult
                        )
                        e_full = p_e.tile([128, 512], MM_DT, tag="e")
                        e_sb = e_full[:, :width]
                        nc.scalar.activation(e_sb[:], s_m[:], AF.Exp)
                        nc.tensor.matmul(
                            po[:],
                            v1[:, kc, :],
                            e_sb[:],
                            start=(kc == 0),
                            stop=(kc == KC - 1),
                        )

                    numz_full = p_nz.tile([DH + 1, 512], F32, tag="nz")
                    numz = numz_full[:, :width]
                    nc.vector.tensor_copy(numz[:], po[:])
                    o_sbn_full = p_o.tile([128, 4, DH], F32, tag="osb")
                    o_sbn = o_sbn_full[:, :nblk, :]
                    for i in range(nblk):
                        pt2 = ps_t2.tile([128, DH + 1], F32, name="pt2")
                        nc.tensor.transpose(
                            pt2[:],
                            numz[:, i * 128 : (i + 1) * 128],
                            ident[0 : DH + 1, 0 : DH + 1],
                        )
                        r_col = p_o.tile([128, 1], F32, tag="rcol")
                        nc.vector.reciprocal(r_col[:], pt2[:, DH : DH + 1])
                        nc.vector.tensor_scalar_mul(
                            o_sbn[:, i, :], pt2[:, 0:DH], r_col[:]
                        )
                    nc.gpsimd.dma_start(
                        o_out[q0 : q0 + width, :].rearrange(
                            "(i p) d -> p i d", p=128
                        ),
                        o_sbn[:],
                    )

                for qb in range(QB):
                    if 1 <= qb < QB - 1:
                        prep(qb + 1)
                    slab = slabs[qb]
                    attend(qb * 512, 512, slab)

            mask_ctx.close()

    nc.finalize()
    return nc


_CACHED_NC = None


def _get_nc():
    global _CACHED_NC
    if _CACHED_NC is None:
        _CACHED_NC = build(bacc.Bacc())
    return _CACHED_NC


def kernel(x, mask, W, b, _trace=False, _tmpdir=None):
    """Full inputs in, full output out. Shards batch across 8 neuron cores."""
    x = np.ascontiguousarray(x, dtype=np.float32)
    mask = np.ascontiguousarray(mask, dtype=np.int32)
    W = np.ascontiguousarray(W, dtype=np.float32)
    b = np.ascontiguousarray(b, dtype=np.float32)
    assert x.shape == (B, S, E) and mask.shape == (B, S, S)

    nc = _get_nc()
    in_maps = [
        {"x": x[c], "mask": mask[c], "W": W, "b": b} for c in range(N_CORES)
    ]
    res = run_bass_kernel_spmd(
        nc, in_maps, list(range(N_CORES)), trace=_trace, tmpdir=_tmpdir
    )
    out = np.stack([res.results[c]["out"] for c in range(N_CORES)])
    if _trace:
        return out, res
    return out


# revision 54
# speedup vs baseline: 1.0146x; 1.0146x over previous
"""Trainium2 Bass kernel for nn_AttentionHead (B=8, S=2048, E=1024, Dh=64).

Sharding: data-parallel over batch B across the 8 NeuronCores (one batch
element per core); W/b replicated; results gathered on host.

Per-core computation, all in "transposed" orientation so no large fp32
matrix ever needs a transpose after the score matmul:
  qkv = x @ W + b; q,k,v = split(qkv); the 1/sqrt(Dh) scale folded into W_q,b_q
  scores^T[k,q] = k^T.T @ q^T          (Dh=64 contraction, PE, fp32r)
  s_m = scores^T * (1-mask^T)          (DVE; bf16 mask, exact 0/1)
  e = exp(s_m)                         (ACT; masked entries -> exp(0)=1,
                                        matching torch masked_fill(mask==1, 0))
  [num^T; Z] = [v | 1]^T @ e           (PE accumulate over k-chunks; the ones
                                        column yields the softmax denominator)
  out = (num * (1/Z))^T                (small PE transpose + DVE reciprocal)

Layout changes: x (8MB) goes through exact fp32 PE transposes; the int32
mask is converted to bf16 t=1-m on GPSIMD and transposed per q-block
either on the PE (first two blocks, whose psum->slab copies land in the
early DVE/ACT slack window) or mostly on the DMA xbar (last two blocks,
keeping copies out of the compute-saturated back half). Matmuls run as
float32r (full PE rate; ~11-bit mantissa) giving ~6e-4 max relative error
end-to-end; set MM_DT = F32 for exact-fp32 matmuls at 4x PE cost.

Cost-model simulated span ~113.5 us/core. Profile shape: DMA-bound to
~60 us (stream gap-free, ends 87.5 us), then DVE/ACT-bound (the 44 us
fp32 mask multiply at DVE's 1x rate plus the 39 us ACT exp pace the
back half), with a ~10 us drain tail.
"""

import os
import sys

sys.path.insert(0, "/opt/trn_rl_repo")

import numpy as np

import concourse.bass as bass
import concourse.tile as tile
from concourse import bacc, mybir
from concourse.masks import make_identity
from concourse.bass_utils import run_bass_kernel_spmd

F32 = mybir.dt.float32
F32R = mybir.dt.float32r
BF16 = mybir.dt.bfloat16
I32 = mybir.dt.int32

B, S, E, DH = 8, 2048, 1024, 64
N_CORES = 8
SCALE = 1.0 / 8.0  # 1/sqrt(DH)

# matmul dtype: F32R = full-rate reduced precision, F32 = exact, 4x slower
MM_DT = F32R

EC = E // 128     # 8   e-chunks
SC = S // 128     # 16  s-chunks (also k-chunks)
QB = S // 512     # 4   q-blocks
KC = SC           # 16  k-chunks per q-block

AF = mybir.ActivationFunctionType
OP = mybir.AluOpType


def build(nc: bass.Bass):
    x_in = nc.dram_tensor("x", [S, E], F32, kind="ExternalInput")
    m_in = nc.dram_tensor("mask", [S, S], I32, kind="ExternalInput")
    w_in = nc.dram_tensor("W", [E, 3 * DH], F32, kind="ExternalInput")
    b_in = nc.dram_tensor("b", [3 * DH], F32, kind="ExternalInput")
    o_out = nc.dram_tensor("out", [S, DH], F32, kind="ExternalOutput")

    trace_sim = bool(os.environ.get("TRN_TRACE_SIM"))
    with tile.TileContext(nc, trace_sim=trace_sim) as tc:
        with (
            tc.tile_pool(name="persist", bufs=1) as persist,
            tc.tile_pool(name="small", bufs=1) as small,
        ):
            # ---- constants / weights -------------------------------------
            ident = persist.tile([128, 128], F32)
            make_identity(nc, ident)
            ident_bf = persist.tile([128, 128], BF16)
            nc.vector.tensor_copy(ident_bf[:], ident[:])

            w_raw = small.tile([128, EC, 3 * DH], F32)
            nc.gpsimd.dma_start(w_raw[:], w_in.rearrange("(o p) d -> p o d", p=128))
            w_sb = persist.tile([128, EC, 3 * DH], MM_DT)
            # fold the 1/sqrt(DH) scale into W_q
            nc.vector.tensor_scalar_mul(w_sb[:, :, 0:DH], w_raw[:, :, 0:DH], SCALE)
            nc.vector.tensor_copy(w_sb[:, :, DH:], w_raw[:, :, DH:])

            b_q_raw = small.tile([64, 1], F32)
            nc.gpsimd.dma_start(b_q_raw[:], b_in[0:64].unsqueeze(-1))
            b_q = persist.tile([64, 1], F32)
            nc.vector.tensor_scalar_mul(b_q[:], b_q_raw[:], SCALE)
            b_k = persist.tile([64, 1], F32)
            nc.gpsimd.dma_start(b_k[:], b_in[64:128].unsqueeze(-1))
            b_v = persist.tile([64, 1], F32)
            nc.gpsimd.dma_start(b_v[:], b_in[128:192].unsqueeze(-1))

            # persistent activations (separate tiles: matmul needs equal base partitions)
            qT = persist.tile([64, S], MM_DT)         # q^T, scale folded
            kT = persist.tile([64, S], MM_DT)         # k^T
            v1 = persist.tile([128, SC, DH + 1], MM_DT)  # v natural + ones col
            ones_col = small.tile([128, 1], F32)
            nc.vector.memset(ones_col[:], 1.0)
            for c in range(SC):
                nc.vector.tensor_copy(v1[:, c, DH : DH + 1], ones_col[:])

            # ---- mask pipeline (spans phase 1 for prefetch) -------------
            from contextlib import ExitStack

            mask_ctx = ExitStack()
            p_m = mask_ctx.enter_context(tc.tile_pool(name="mstage", bufs=4))
            p_tnat = mask_ctx.enter_context(tc.tile_pool(name="tnat", bufs=4))
            p_slab = mask_ctx.enter_context(tc.tile_pool(name="slab", bufs=2))
            ps_tm = mask_ctx.enter_context(
                tc.tile_pool(name="ps_tm", bufs=2, space="PSUM")
            )

            slabs = {}

            def prep(qb, pe_chunks=(3,)):
                """mask rows for q-block -> t^T slab [k_in, k_out, q].

                Chunks in pe_chunks are transposed on the PE instead of the
                DMA xbar (keeps them off the serialized DMA stream)."""
                slab = p_slab.tile([128, SC, 512], BF16)
                m_tiles = []
                for mc in range(4):
                    m_i32 = p_m.tile([128, S], I32)
                    nc.sync.dma_start(
                        m_i32[:],
                        m_in[qb * 512 + mc * 128 : qb * 512 + (mc + 1) * 128, :],
                    )
                    m_tiles.append(m_i32)
                for mc in range(4):
                    t_nat = p_tnat.tile([128, S], BF16)
                    nc.gpsimd.tensor_scalar(
                        t_nat[:], m_tiles[mc][:], -1, 1, OP.mult, OP.add
                    )
                    if mc not in pe_chunks:
                        nc.sync.dma_start_transpose(
                            slab[:, :, mc * 128 : (mc + 1) * 128], t_nat[:]
                        )
                    else:
                        # PE path: offload the 4th transpose from the DMA xbar
                        for g in range(4):
                            pstm = ps_tm.tile([128, 512], BF16, name="pstm")
                            for j4 in range(4):
                                kc_t = g * 4 + j4
                                nc.tensor.transpose(
                                    pstm[:, j4 * 128 : (j4 + 1) * 128],
                                    t_nat[:, kc_t * 128 : (kc_t + 1) * 128],
                                    ident_bf[:],
                                )
                            cp = nc.scalar.copy if (g + mc) % 2 else nc.vector.tensor_copy
                            cp(
                                slab[:, g * 4 : (g + 1) * 4,
                                     mc * 128 : (mc + 1) * 128],
                                pstm[:].rearrange("p (j f) -> p j f", j=4),
                            )
                slabs[qb] = slab

            # ---- phase 1: x^T, qkv (mask prefetch interleaved) ----------
            with (
                tc.tile_pool(name="xnat", bufs=2) as p_xnat,
                tc.tile_pool(name="xT", bufs=2) as p_xT,
                tc.tile_pool(name="vT", bufs=1) as p_vT,
                tc.tile_pool(name="ps_t", bufs=2, space="PSUM") as ps_t,
                tc.tile_pool(name="ps_mm", bufs=2, space="PSUM") as ps_mm,
            ):
                vT = p_vT.tile([64, S], F32)
                for nt in range(QB):
                    sl = slice(nt * 512, (nt + 1) * 512)
                    x_T = p_xT.tile([128, EC, 512], MM_DT)
                    x_nat4 = p_xnat.tile([128, 4, E], F32)
                    for h in range(2):  # two 1MB halves for finer pipelining
                        nc.sync.dma_start(
                            x_nat4[:, h * 2 : (h + 1) * 2, :],
                            x_in[nt * 512 + h * 256 : nt * 512 + (h + 1) * 256, :]
                            .rearrange("(c p) e -> p c e", p=128),
                        )
                    for c4 in range(4):
                        c = nt * 4 + c4
                        for g in range(2):  # 4 transposes -> 1 psum tile
                            pst = ps_t.tile([128, 512], F32, name="pst")
                            for j4 in range(4):
                                j = g * 4 + j4
                                nc.tensor.transpose(
                                    pst[:, j4 * 128 : (j4 + 1) * 128],
                                    x_nat4[:, c4, j * 128 : (j + 1) * 128],
                                    ident[:],
                                )
                            # copy psum -> x_T (rounds to f32r); alternate engines
                            dst = x_T[:, g * 4 : (g + 1) * 4, c4 * 128 : (c4 + 1) * 128]
                            src = pst[:].rearrange("p (j f) -> p j f", j=4)
                            if c % 2 == 0:
                                nc.vector.tensor_copy(dst, src)
                            else:
                                nc.scalar.copy(dst, src)

                    for dst, lo, bias in (
                        (qT, 0, b_q),
                        (kT, DH, b_k),
                        (vT, 2 * DH, b_v),
                    ):
                        ps_p = ps_mm.tile([64, 512], F32, name="ps_p")
                        for j in range(EC):
                            nc.tensor.matmul(
                                ps_p[:],
                                w_sb[:, j, lo : lo + DH],
                                x_T[:, j, :],
                                start=(j == 0),
                                stop=(j == EC - 1),
                            )
                        nc.vector.tensor_scalar_add(dst[:, sl], ps_p[:], bias[:])

                    # v natural for this 512-slice
                    for c4 in range(4):
                        c = nt * 4 + c4
                        psv = ps_t.tile([128, 512], F32, name="pst")
                        nc.tensor.transpose(
                            psv[:, 0:64],
                            vT[:, c * 128 : (c + 1) * 128],
                            ident[0:64, 0:64],
                        )
                        nc.vector.tensor_copy(v1[:, c, 0:DH], psv[:, 0:64])

                # mask prefetch for the first two q-blocks after all x traffic.
                # These two transpose fully on the PE: their psum->slab copies
                # land in the early window where DVE/ACT are idle, and the DMA
                # stream (which gates the last q-block's compute) shortens.
                prep(0, pe_chunks=(0, 1, 2, 3))
                prep(1, pe_chunks=(0, 1, 2, 3))

            # ---- phase 3: attention -------------------------------------
            with (
                tc.tile_pool(name="sm", bufs=4) as p_sm,
                tc.tile_pool(name="e", bufs=4) as p_e,
                tc.tile_pool(name="nz", bufs=2) as p_nz,
                tc.tile_pool(name="osb", bufs=2) as p_o,
                tc.tile_pool(name="ps_s", bufs=3, space="PSUM") as ps_s,
                tc.tile_pool(name="ps_o", bufs=2, space="PSUM") as ps_o,
                tc.tile_pool(name="ps_t2", bufs=1, space="PSUM") as ps_t2,
            ):
                def attend(q0, width, slab):
                    """kc-loop + normalize + store for q columns [q0, q0+width)."""
                    nblk = width // 128
                    po_full = ps_o.tile([DH + 1, 512], F32, name="po")
                    po = po_full[:, :width]
                    for kc in range(KC):
                        pss_full = ps_s.tile([128, 512], F32, name="pss")
                        pss = pss_full[:, :width]
                        nc.tensor.matmul(
                            pss[:],
                            kT[:, kc * 128 : (kc + 1) * 128],
                            qT[:, q0 : q0 + width],
                            start=True,
                            stop=True,
                        )
                        s_m_full = p_sm.tile([128, 512], F32, tag="sm")
                        s_m = s_m_full[:, :width]
                        nc.vector.tensor_tensor(
                            s_m[:], pss[:],
                            slab[:, kc, q0 % 512 : q0 % 512 + width], OP.mult
                        )
                        e_full = p_e.tile([128, 512], MM_DT, tag="e")
                        e_sb = e_full[:, :width]
                        nc.scalar.activation(e_sb[:], s_m[:], AF.Exp)
                        nc.tensor.matmul(
                            po[:],
                            v1[:, kc, :],
                            e_sb[:],
                            start=(kc == 0),
                            stop=(kc == KC - 1),
                        )

                    numz_full = p_nz.tile([DH + 1, 512], F32, tag="nz")
                    numz = numz_full[:, :width]
                    nc.vector.tensor_copy(numz[:], po[:])
                    o_sbn_full = p_o.tile([128, 4, DH], F32, tag="osb")
                    o_sbn = o_sbn_full[:, :nblk, :]
                    for i in range(nblk):
                        pt2 = ps_t2.tile([128, DH + 1], F32, name="pt2")
                        nc.tensor.transpose(
                            pt2[:],
                            numz[:, i * 128 : (i + 1) * 128],
                            ident[0 : DH + 1, 0 : DH + 1],
                        )
                        r_col = p_o.tile([128, 1], F32, tag="rcol")
                        nc.vector.reciprocal(r_col[:], pt2[:, DH : DH + 1])
                        # per-partition scale on ACT (Copy is in every table
                        # set, no swap); frees DVE in its saturated window
                        nc.scalar.activation(
                            o_sbn[:, i, :], pt2[:, 0:DH], AF.Copy,
                            scale=r_col[:],
                        )
                    nc.gpsimd.dma_start(
                        o_out[q0 : q0 + width, :].rearrange(
                            "(i p) d -> p i d", p=128
                        ),
                        o_sbn[:],
                    )

                for qb in range(QB):
                    if 1 <= qb < QB - 1:
                        prep(qb + 1)
                    slab = slabs[qb]
                    attend(qb * 512, 512, slab)

            mask_ctx.close()

    nc.finalize()
    return nc


_CACHED_NC = None


def _get_nc():
    global _CACHED_NC
    if _CACHED_NC is None:
        _CACHED_NC = build(bacc.Bacc())
    return _CACHED_NC


def kernel(x, mask, W, b, _trace=False, _tmpdir=None):
    """Full inputs in, full output out. Shards batch across 8 neuron cores."""
    x = np.ascontiguousarray(x, dtype=np.float32)
    mask = np.ascontiguousarray(mask, dtype=np.int32)
    W = np.ascontiguousarray(W, dtype=np.float32)
    b = np.ascontiguousarray(b, dtype=np.float32)
    assert x.shape == (B, S, E) and mask.shape == (B, S, S)

    nc = _get_nc()
    in_maps = [
        {"x": x[c], "mask": mask[c], "W": W, "b": b} for c in range(N_CORES)
    ]
    res = run_bass_kernel_spmd(
        nc, in_maps, list(range(N_CORES)), trace=_trace, tmpdir=_tmpdir
    )
    out = np.stack([res.results[c]["out"] for c in range(N_CORES)])
    if _trace:
        return out, res
    return out
